# revision 1
# baseline (speedup 1.0000x reference)
"""CRF log-partition minus gold-path score.

Forward algorithm reformulated as an associative product of per-step
transition operators. In log space each step is a log-matmul by
M_t[i,j] = transitions[i,j] + feats[t,i]; only the *total* product is
needed (not the per-step prefix), so the 2M-step sequential scan becomes
a pairwise tree reduction over 5x5 operators carried in scaled
probability space (matrices renormalized to max=1 per level, with the
log-scales accumulated separately in float64).
"""

import numpy as np

NTAGS = 5
START, STOP = 3, 4
NEG = -10000.0


def _log_matmul_chain(feats: np.ndarray, transitions: np.ndarray):
    """Return (P, logscale): P is the 5x5 product M_{T-1} x ... x M_0 in
    probability space scaled so max(P)=1, logscale the accumulated log-scale."""
    T = feats.shape[0]
    trans = transitions.astype(np.float32)
    # M[t, i, j] = trans[i, j] + feats[t, i]
    M = trans[None, :, :] + feats[:, :, None].astype(np.float32)
    s = M.max(axis=(1, 2))  # [T] per-step scale
    P = np.exp(M - s[:, None, None])
    logscale = s.astype(np.float64).sum()

    while P.shape[0] > 1:
        n = P.shape[0]
        m = n - (n % 2)
        left = P[1:m:2]   # later steps -> left factor
        right = P[0:m:2]
        # batched 5x5 matmul without per-item BLAS dispatch
        C = (left[:, :, :, None] * right[:, None, :, :]).sum(axis=2)
        if n % 2:
            C = np.concatenate([C, P[m:]], axis=0)
        sc = C.max(axis=(1, 2))
        sc = np.where(sc > 0, sc, 1.0).astype(np.float32)
        C /= sc[:, None, None]
        logscale += np.log(sc.astype(np.float64)).sum()
        P = C
    return P[0], logscale


def kernel(feats: np.ndarray, tags: np.ndarray, transitions: np.ndarray) -> np.ndarray:
    feats = np.asarray(feats)
    tags_i = np.asarray(tags).astype(np.int64)
    trans = np.asarray(transitions).astype(np.float32)
    T = feats.shape[0]

    # ---- forward algorithm (log partition) via associative reduction ----
    P, logscale = _log_matmul_chain(feats, trans)
    # init vector: onehot at START in probability space (exp(-10000) == 0)
    u = P[:, START].astype(np.float64)  # P @ onehot(START)
    w = np.exp(trans[STOP].astype(np.float64))  # final transition into STOP
    alpha = np.log((w * u).sum()) + logscale

    # ---- gold path score ----
    trans64 = trans.astype(np.float64)
    prev = np.concatenate([np.array([START], dtype=np.int64), tags_i[:-1]])
    trans_score = trans64[tags_i, prev].sum()
    emit_score = feats.astype(np.float64)[np.arange(T), tags_i].sum()
    gold = trans_score + emit_score + trans64[STOP, tags_i[-1]]

    return np.asarray(alpha - gold, dtype=np.float32)



# revision 2
# speedup vs baseline: 5370.2347x; 5370.2347x over previous
"""CRF log-partition minus gold-path score, on 8 trn2 NeuronCores.

alpha (the log partition) is computed in probability space as a product of
2M transition operators. Mid-chain, only tags {0,1,2} are reachable (the
transition row into START and column out of STOP are -1e4, and any path
entering STOP mid-chain is stuck there), so each step is a 3x3 operator
A_t = E^T diag(e_t) with E = exp(transitions[:3,:3] - mu), e_t =
exp(feats[t,:3]). A constant per-step scale mu (calibrated on the first 1k
steps) keeps magnitudes in f32/bf16 range over a chunk without per-step
renormalization.

Device kernel (SPMD over 8 cores, 250k steps each): the core's steps are
split into 20916 independent chunks of <=12 steps that advance in lockstep.
42 chunks ride one 126x498 matmul per scan round (block-diagonal stationary
of 42 E-copies, loaded once and reused via ldweights=False), and the
per-step diag(e_t) multiply is fused into the PSUM->SBUF copy on VectorE.
Three interleaved waves keep PE/DVE busy. e = exp(feats) is computed on the
host and shipped as bf16 (1.5 MB/core). The host combines the 167k 3x3
chunk products in f64 (tree with rescaling) and fixes the START/STOP
boundary terms exactly.

The gold-path score is a pure gather/sum, done on the host in f64.
Any failure in the device path falls back to a pure-numpy tree reduction.
"""

from contextlib import ExitStack

import numpy as np

NTAGS = 5
START, STOP = 3, 4
NTE = 3

T_FULL = 2_000_000
NCORES = 8
T_CORE = T_FULL // NCORES
L = 12
CBLK = 42
G = 498
NB = CBLK * G
NLONG = NB - (NB * L - T_CORE)
W = 3
GW = G // W
P = CBLK * NTE
FD = GW * NTE
TSPLIT = 4
NWARM = 3
HDR = P + NTE

_PROGRAM = None


def _build_program():
    from concourse import bass, mybir

    F32 = mybir.dt.float32
    BF16 = mybir.dt.bfloat16

    def matmul_noload(pe, out, lhsT, rhs):
        ifmap_ap = pe.lower_ap(rhs.opt({0}), opt=False)
        weights_ap = pe.lower_ap(lhsT.opt({0}), opt=False, for_matmul_weights=True)
        out_ap = pe.lower_ap(out)
        return pe.add_instruction(mybir.InstMatmult(
            name=pe.bass.get_next_instruction_name(),
            replication_resolution=0, replication_shift_amnt=0,
            replication_num_rows=0,
            start_tensor_calc=True, stop_tensor_calc=True,
            ins=[ifmap_ap, weights_ap], outs=[out_ap],
            tile_position=(0, 0), tile_size=(128, 128),
            ldweights=False,
        ))

    nc = bass.Bass(enable_partition_id=False)
    blob_in = nc.declare_dram_parameter("blob", [P, HDR + L * G], BF16, isOutput=False)
    out_last = nc.declare_dram_parameter("out_last", [P, G * NTE], BF16, isOutput=True)
    out_prev = nc.declare_dram_parameter("out_prev", [P, G * NTE], BF16, isOutput=True)

    ctx = ExitStack()
    with ctx:
        blob_sb = ctx.enter_context(nc.sbuf_tensor("blob_sb", [P, HDR + L * G], BF16))
        ol_sb = ctx.enter_context(nc.sbuf_tensor("ol_sb", [P, G * NTE], BF16))
        op_sb = ctx.enter_context(nc.sbuf_tensor("op_sb", [P, G * NTE], BF16))
        rhs = [
            [
                ctx.enter_context(nc.sbuf_tensor(f"rhs_{w}_{p}", [P, GW, NTE], BF16))
                for p in range(2)
            ]
            for w in range(W)
        ]
        ps = [
            [
                ctx.enter_context(nc.psum_tensor(f"ps_{w}_{p}", [P, FD], F32))
                for p in range(2)
            ]
            for w in range(W)
        ]
        ps_warm = ctx.enter_context(nc.psum_tensor("ps_warm", [P, FD], F32))
        dma_in_sem = ctx.enter_context(nc.semaphore("dma_in_sem"))
        rhs_sem = ctx.enter_context(nc.semaphore("rhs_sem"))
        mm_sem = ctx.enter_context(nc.semaphore("mm_sem"))
        done_sem = ctx.enter_context(nc.semaphore("done_sem"))
        block = ctx.enter_context(nc.Block())

        ebd_sb = blob_sb[:, :P]
        eye_sb = blob_sb[:, P : P + NTE]

        def e_bc(t, w):
            base = HDR + t * G + w * GW
            return blob_sb[:, base : base + GW].unsqueeze(2).broadcast_to(
                [P, GW, NTE]
            )

        SPLIT = HDR + TSPLIT * G

        @block.sync
        def _(eng):
            eng.dma_start(out=blob_sb[:, :SPLIT], in_=blob_in[:, :SPLIT]).then_inc(
                dma_in_sem, 16
            )
            eng.dma_start(out=blob_sb[:, SPLIT:], in_=blob_in[:, SPLIT:]).then_inc(
                dma_in_sem, 16
            )

        @block.vector
        def _(eng):
            eng.wait_ge(dma_in_sem, 16)
            for w in range(W):
                eng.tensor_tensor(
                    rhs[w][0][:],
                    e_bc(0, w),
                    eye_sb.unsqueeze(1).broadcast_to([P, GW, NTE]),
                    mybir.AluOpType.mult,
                ).then_inc(rhs_sem, 1)
            for t in range(1, L):
                if t == TSPLIT:
                    eng.wait_ge(dma_in_sem, 32)
                for w in range(W):
                    eng.wait_ge(mm_sem, (t - 1) * W + w + 1)
                    eng.tensor_tensor(
                        rhs[w][t % 2][:],
                        ps[w][(t - 1) % 2][:].rearrange("p (g j) -> p g j", j=NTE),
                        e_bc(t, w),
                        mybir.AluOpType.mult,
                    ).then_inc(rhs_sem, 1)

        @block.scalar
        def _(eng):
            for w in range(W):
                eng.wait_ge(mm_sem, (L - 2) * W + w + 1)
                eng.activation(
                    op_sb[:, w * FD : (w + 1) * FD],
                    ps[w][(L - 2) % 2][:],
                    mybir.ActivationFunctionType.Copy,
                ).then_inc(done_sem, 1)
            for w in range(W):
                eng.wait_ge(mm_sem, (L - 1) * W + w + 1)
                eng.activation(
                    ol_sb[:, w * FD : (w + 1) * FD],
                    ps[w][(L - 1) % 2][:],
                    mybir.ActivationFunctionType.Copy,
                ).then_inc(done_sem, 1)

        @block.tensor
        def _(eng):
            eng.wait_ge(dma_in_sem, 16)
            eng.matmul(ps_warm[:, :P], ebd_sb, ebd_sb, start=True, stop=True)
            for _ in range(NWARM - 1):
                matmul_noload(eng, ps_warm[:, :P], ebd_sb, ebd_sb)
            for t in range(L):
                for w in range(W):
                    eng.wait_ge(rhs_sem, t * W + w + 1)
                    matmul_noload(
                        eng,
                        ps[w][t % 2][:],
                        ebd_sb,
                        rhs[w][t % 2][:].rearrange("p g j -> p (g j)"),
                    ).then_inc(mm_sem, 1)

        @block.gpsimd
        def _(eng):
            for w in range(W):
                eng.wait_ge(done_sem, w + 1)
                eng.dma_start(
                    out=out_prev[:, w * FD : (w + 1) * FD],
                    in_=op_sb[:, w * FD : (w + 1) * FD],
                ).then_inc(dma_in_sem, 16)
            for w in range(W):
                eng.wait_ge(done_sem, W + w + 1)
                eng.dma_start(
                    out=out_last[:, w * FD : (w + 1) * FD],
                    in_=ol_sb[:, w * FD : (w + 1) * FD],
                ).then_inc(dma_in_sem, 16)
            eng.wait_ge(dma_in_sem, 32 + 16 * 2 * W)

    return nc


def _calibrate_mu(feats, transitions, n=1000):
    E = np.exp(transitions.astype(np.float64))
    v = np.ones(NTAGS)
    acc = 0.0
    for t in range(n):
        v = (E.T * np.exp(feats[t].astype(np.float64))) @ v
        m = v.max()
        v /= m
        acc += np.log(m)
    return acc / n


def _prep_inputs(feats, transitions, mu):
    import ml_dtypes

    def bf16(x):
        return np.asarray(x, dtype=ml_dtypes.bfloat16)

    E3 = np.exp(transitions[:NTE, :NTE].astype(np.float64) - mu).astype(np.float32)
    hdr = np.zeros((P, HDR), np.float32)
    for c in range(CBLK):
        hdr[NTE * c : NTE * (c + 1), NTE * c : NTE * (c + 1)] = E3
        hdr[NTE * c : NTE * (c + 1), P : P + NTE] = np.eye(NTE, dtype=np.float32)
    hdr16 = bf16(hdr)

    f = np.asarray(feats, np.float32).reshape(NCORES, T_CORE, NTAGS)
    in_maps = []
    for k in range(NCORES):
        e3 = np.exp(f[k, :, :NTE].astype(np.float32))
        e_full = np.ones((NB, L, NTE), np.float32)
        e_full[:NLONG] = e3[: NLONG * L].reshape(NLONG, L, NTE)
        e_full[NLONG:, : L - 1] = e3[NLONG * L :].reshape(NB - NLONG, L - 1, NTE)
        er = np.ascontiguousarray(
            e_full.reshape(CBLK, G, L, NTE).transpose(0, 3, 2, 1)
        ).reshape(P, L * G)
        in_maps.append({"blob": np.concatenate([hdr16, bf16(er)], axis=1)})
    return in_maps


def _extract_chunks(res_last, res_prev):
    def to_mats(a):
        u = np.asarray(a, np.float64).reshape(CBLK, NTE, W, GW, NTE)
        return u.transpose(0, 2, 3, 1, 4).reshape(NB, NTE, NTE)

    out = to_mats(res_last)
    out[NLONG:] = to_mats(res_prev)[NLONG:]
    return out


def _combine_host(mats, feats, transitions, mu):
    T = feats.shape[0]
    tr = np.asarray(transitions, np.float64)
    E3 = np.exp(tr[:NTE, :NTE] - mu)

    # exact last chunk: z^T = rowSTART(A_{T-1}) * prod(A_t desc)
    last_len = L - 1 if NB > NLONG else L
    off_last = T - last_len
    e3 = np.exp(feats[off_last:, :NTE].astype(np.float64))
    z = np.exp(tr[:NTE, START] - mu) * e3[-1]
    for t in range(last_len - 2, -1, -1):
        z = e3[t] * (E3 @ z)
    Z = np.zeros((NTE, NTE))
    Z[0] = z

    def local_off(local):
        return L * local if local < NLONG else L * NLONG + (L - 1) * (local - NLONG)

    Pm = np.concatenate([mats[:-1], Z[None]], axis=0)
    bad = ~np.isfinite(Pm).all(axis=(1, 2)) | (np.abs(Pm).max(axis=(1, 2)) <= 0)
    if bad.any():
        for m in np.nonzero(bad)[0]:
            if m == len(Pm) - 1:
                raise FloatingPointError("host z chunk non-finite")
            k, local = divmod(int(m), NB)
            off = k * T_CORE + local_off(local)
            ln = L if local < NLONG else L - 1
            fb = np.exp(feats[off : off + ln, :NTE].astype(np.float64))
            U = np.eye(NTE)
            for t in range(ln):
                U = E3.T @ (fb[t][:, None] * U)
            Pm[m] = U
    n = np.abs(Pm).max(axis=(1, 2))
    Pm = Pm / n[:, None, None]
    ls = np.log(n).sum()
    while Pm.shape[0] > 1:
        nn_ = Pm.shape[0]
        m = nn_ - (nn_ % 2)
        C = np.einsum("bik,bkj->bij", Pm[1:m:2], Pm[0:m:2])
        if nn_ % 2:
            C = np.concatenate([C, Pm[m:]], 0)
        sc = np.abs(C).max(axis=(1, 2))
        sc = np.where(sc > 0, sc, 1.0)
        C /= sc[:, None, None]
        ls += np.log(sc).sum()
        Pm = C
    val = float(Pm[0][0] @ np.exp(tr[STOP, :NTE]))
    if not (np.isfinite(val) and val > 0):
        raise FloatingPointError("combine produced non-positive value")
    return float(np.log(val) + ls + T * mu)


def _gold_score(feats, tags, transitions):
    T = feats.shape[0]
    tags_i = np.asarray(tags).astype(np.int64)
    tr = np.asarray(transitions, np.float64)
    prev = np.concatenate([np.array([START], np.int64), tags_i[:-1]])
    trans_score = tr[tags_i, prev].sum()
    emit_score = np.asarray(feats, np.float64)[np.arange(T), tags_i].sum()
    return trans_score + emit_score + tr[STOP, tags_i[-1]]


def _alpha_device(feats, transitions):
    global _PROGRAM
    from concourse.bass_utils import run_bass_kernel_spmd

    if _PROGRAM is None:
        _PROGRAM = _build_program()
    mu = _calibrate_mu(feats, transitions)
    in_maps = _prep_inputs(feats, transitions, mu)
    res = run_bass_kernel_spmd(_PROGRAM, in_maps, list(range(NCORES)))
    mats = np.concatenate(
        [
            _extract_chunks(res.results[k]["out_last"], res.results[k]["out_prev"])
            for k in range(NCORES)
        ],
        axis=0,
    )
    return _combine_host(mats, feats, transitions, mu)


def _alpha_numpy(feats, transitions):
    """Fallback: pairwise tree reduction over 5x5 operators (pure numpy)."""
    trans = transitions.astype(np.float32)
    M = trans[None, :, :] + feats[:, :, None].astype(np.float32)
    s = M.max(axis=(1, 2))
    Pr = np.exp(M - s[:, None, None])
    ls = s.astype(np.float64).sum()
    while Pr.shape[0] > 1:
        n = Pr.shape[0]
        m = n - (n % 2)
        C = (Pr[1:m:2][:, :, :, None] * Pr[0:m:2][:, None, :, :]).sum(axis=2)
        if n % 2:
            C = np.concatenate([C, Pr[m:]], axis=0)
        sc = C.max(axis=(1, 2))
        sc = np.where(sc > 0, sc, 1.0).astype(np.float32)
        C /= sc[:, None, None]
        ls += np.log(sc.astype(np.float64)).sum()
        Pr = C
    u = Pr[0][:, START].astype(np.float64)
    w = np.exp(trans[STOP].astype(np.float64))
    return float(np.log((w * u).sum()) + ls)


def kernel(feats, tags, transitions):
    feats = np.asarray(feats, np.float32)
    tags = np.asarray(tags)
    transitions = np.asarray(transitions, np.float32)
    T = feats.shape[0]

    alpha = None
    if T == T_FULL:
        try:
            alpha = _alpha_device(feats, transitions)
        except Exception:
            alpha = None
    if alpha is None:
        alpha = _alpha_numpy(feats, transitions)

    gold = _gold_score(feats, tags, transitions)
    return np.asarray(alpha - gold, dtype=np.float32)
